# revision 1
# baseline (speedup 1.0000x reference)
"""DeformConv1d Trainium2 Bass kernel, v2.

Same algorithm as v1 (see kernel.py docstring) with:
  - all 5 taps (k) merged into single wide DVE instructions using
    overlapping access patterns (the per-tap sample centers differ by 1
    element, so a [[1,5],[1,LT]] AP view of d1/BASE covers all taps);
  - LT=1024 l-tiles;
  - even/odd phase copies of d1 so every shifted fp16 view starts
    4B-aligned (keeps the DVE 2x packed mode).
"""

import numpy as np

B, CIN, COUT, L, K, G = 8, 256, 256, 8192, 5, 4
PAD = 2
JMIN, JMAX = -3, 3
MARG = 8
LT = 512
NLT = L // LT
NCH = 2
CPG = CIN // G

_CACHE = {}
TRACE = False
LAST_EXEC_NS = None


def _pack_weights(w_off, b_off, weight, bias):
    f16 = np.float16
    w_off_r = w_off.reshape(2, CIN, K, CPG, K)
    # conv group of output (d, c) is co//640 = 2*d + c//128; its 64 input
    # channels are group*64..+64, living in xpad[d] partitions (c//128)*64..+64
    wofflhsT = np.zeros((128, 2 * K * NCH * K, 128), f16)
    for d in range(2):
        for k in range(K):
            for ch in range(NCH):
                for kp in range(K):
                    t = ((d * K + k) * NCH + ch) * K + kp
                    blk = np.zeros((128, 128), np.float32)
                    sub = w_off_r[d, ch * 128:(ch + 1) * 128, k, :, kp]  # (c128, ci64)
                    blk[ch * 64:(ch + 1) * 64, :] = sub.T
                    wofflhsT[:, t, :] = blk.astype(f16)
    w_r = weight.reshape(COUT, CPG, K)
    wfinlhsT = np.zeros((128, K * NCH, 128), f16)
    for k in range(K):
        for ch in range(NCH):
            blk = np.zeros((128, 128), np.float32)
            for half in range(2):
                g = ch * 2 + half
                sub = w_r[g * 64:(g + 1) * 64, :, k]
                blk[half * 64:(half + 1) * 64, half * 64:(half + 1) * 64] = sub.T
            wfinlhsT[:, k * NCH + ch, :] = blk.astype(f16)
    b_off_r = b_off.reshape(2, CIN, K)
    boffs = np.zeros((128, NCH, 2 * K), np.float32)
    for ch in range(NCH):
        for d in range(2):
            for k in range(K):
                boffs[:, ch, d * K + k] = b_off_r[d, ch * 128:(ch + 1) * 128, k]
    bfin = bias.reshape(NCH, 128).T.astype(np.float32).copy()
    p = np.arange(128)
    ones_sm = (p[:, None] % 64 == p[None, :] % 64).astype(f16)
    return (np.ascontiguousarray(wofflhsT), np.ascontiguousarray(wfinlhsT),
            np.ascontiguousarray(boffs), np.ascontiguousarray(bfin),
            np.ascontiguousarray(ones_sm))


def _build(nc):
    import concourse.bass as bass
    import concourse.tile as tile
    import concourse.mybir as mybir
    from concourse.mybir import AluOpType as alu

    def ov(slice_ap, count0, count1):
        """Overlapping [[1,count0],[1,count1]] view anchored at slice_ap's start."""
        return bass.AP(tensor=slice_ap.tensor, offset=slice_ap.offset,
                       ap=[list(slice_ap.ap[0]), [1, count0], [1, count1]])

    f16 = mybir.dt.float16
    f32 = mybir.dt.float32
    AF = mybir.ActivationFunctionType

    x_d = nc.dram_tensor("x", [CIN, L], f32, kind="ExternalInput")
    woff_d = nc.dram_tensor("wofflhsT", [128, 2 * K * NCH * K, 128], f16, kind="ExternalInput")
    wfin_d = nc.dram_tensor("wfinlhsT", [128, K * NCH, 128], f16, kind="ExternalInput")
    boffs_d = nc.dram_tensor("boffs", [128, NCH, 2 * K], f32, kind="ExternalInput")
    bfin_d = nc.dram_tensor("bfin", [128, NCH], f32, kind="ExternalInput")
    ones_d = nc.dram_tensor("ones_sm", [128, 128], f16, kind="ExternalInput")
    out_d = nc.dram_tensor("out", [CIN, L], f32, kind="ExternalOutput")

    XW = L + 2 * MARG
    NSUB = LT // 512  # psum sub-tiles per l-tile

    with tile.TileContext(nc) as tc:
        with (
            tc.tile_pool(name="consts", bufs=1) as consts,
            tc.tile_pool(name="resid", bufs=1) as resid,
            tc.tile_pool(name="stage", bufs=1) as stage,
            tc.tile_pool(name="work", bufs=2) as work,
            tc.tile_pool(name="psc", bufs=4, space="PSUM") as psc,
            tc.tile_pool(name="pss", bufs=2, space="PSUM") as pss,
            tc.tile_pool(name="psf", bufs=2, space="PSUM") as psf,
        ):
            w_sb = consts.tile([128, 2 * K * NCH * K, 128], f16, name="w_sb", tag="w_sb")
            nc.gpsimd.dma_start(out=w_sb, in_=woff_d[:, :, :])
            wfin_sb = consts.tile([128, K * NCH, 128], f16, name="wfin_sb", tag="wfin_sb")
            nc.gpsimd.dma_start(out=wfin_sb, in_=wfin_d[:, :, :])
            boff_sb = consts.tile([128, NCH, 2 * K], f32, name="boff_sb", tag="boff_sb")
            nc.gpsimd.dma_start(out=boff_sb, in_=boffs_d[:, :, :])
            bfin_sb = consts.tile([128, NCH], f32, name="bfin_sb", tag="bfin_sb")
            nc.gpsimd.dma_start(out=bfin_sb, in_=bfin_d[:, :])
            ones_sb = consts.tile([128, 128], f16, name="ones_sb", tag="ones_sb")
            nc.gpsimd.dma_start(out=ones_sb, in_=ones_d[:, :])

            xpad = []
            for ch in range(NCH):
                xp = resid.tile([128, XW], f16, name=f"xpad{ch}", tag=f"xpad{ch}")
                nc.vector.memset(xp, 0.0)
                xpad.append(xp)
            SST = 4096
            for ch in range(NCH):
                for i in range(L // SST):
                    st = stage.tile([128, SST], f32, name="xstage", tag="xstage")
                    nc.gpsimd.dma_start(
                        out=st, in_=x_d[ch * 128:(ch + 1) * 128, i * SST:(i + 1) * SST])
                    nc.scalar.activation(
                        out=xpad[ch][:, MARG + i * SST:MARG + (i + 1) * SST],
                        in_=st, func=AF.Copy, bias=0.0, scale=1.0)

            for lt in range(NLT):
                l0 = lt * LT
                # d1 tile covers t in [l0-6, l0+LT+6); phase B shifted by 1.
                D1LO = l0 - 6
                D1W = LT + 12
                BLO = l0 - 2            # base covers m in [l0-2, l0+LT+2)
                d1A, d1B, baset = [], [], []
                for ch in range(NCH):
                    dA = work.tile([128, D1W + 2], f16, name=f"d1A_{ch}", tag=f"d1A_{ch}", bufs=1)
                    nc.gpsimd.tensor_tensor(
                        out=dA[:, 0:D1W + 1],
                        in0=xpad[ch][:, MARG + D1LO + 1:MARG + D1LO + 1 + D1W + 1],
                        in1=xpad[ch][:, MARG + D1LO:MARG + D1LO + D1W + 1],
                        op=alu.subtract)
                    # phase-B copy: d1B[i] = d1A[i+1] (so odd A-offsets are even in B)
                    dB = work.tile([128, D1W + 1], f16, name=f"d1B_{ch}", tag=f"d1B_{ch}", bufs=1)
                    nc.gpsimd.tensor_copy(out=dB[:, 0:D1W], in_=dA[:, 1:D1W + 1])
                    span = LT + 4

                    def d1s(j, dA=dA):
                        s = (BLO + j) - D1LO
                        return dA[:, s:s + span]

                    t1 = work.tile([128, span], f16, name=f"t1_{ch}", tag=f"t1_{ch}", bufs=1)
                    nc.gpsimd.tensor_tensor(out=t1, in0=d1s(-3), in1=d1s(2), op=alu.subtract)
                    t2 = work.tile([128, span], f16, name=f"t2_{ch}", tag=f"t2_{ch}", bufs=1)
                    nc.gpsimd.tensor_tensor(out=t2, in0=d1s(-2), in1=d1s(1), op=alu.subtract)
                    bs = work.tile([128, span], f16, name=f"base_{ch}", tag=f"base_{ch}", bufs=1)
                    nc.vector.scalar_tensor_tensor(
                        out=bs, in0=t1, scalar=2.0, in1=t2, op0=alu.mult, op1=alu.add)
                    bs2 = work.tile([128, span], f16, name=f"base2_{ch}", tag=f"base2_{ch}", bufs=1)
                    nc.gpsimd.tensor_tensor(
                        out=bs2, in0=bs,
                        in1=xpad[ch][:, MARG + BLO:MARG + BLO + span], op=alu.add)
                    d1A.append(dA)
                    d1B.append(dB)
                    baset.append(bs2)

                # conv_off + drains; off/exp merged [128, K, LT]
                off_t, exp_t = [], []
                for ch in range(NCH):
                    off_t.append(work.tile([128, K, LT], f16, name=f"off_{ch}", tag=f"off_{ch}", bufs=2))
                    exp_t.append(work.tile([128, K, LT], f16, name=f"exp_{ch}", tag=f"exp_{ch}", bufs=2))
                for d in range(2):
                    for k in range(K):
                        for ch in range(NCH):
                            ps = psc.tile([128, LT], f32, name="pconv", tag="pconv")
                            for sub in range(NSUB):
                                for kp in range(K):
                                    t = ((d * K + k) * NCH + ch) * K + kp
                                    s0 = MARG + l0 + sub * 512 + kp - 2
                                    nc.tensor.matmul(
                                        ps[:, sub * 512:(sub + 1) * 512],
                                        lhsT=w_sb[:, t, :],
                                        rhs=xpad[d][:, s0:s0 + 512],
                                        start=(kp == 0), stop=(kp == K - 1))
                            if d == 0:
                                nc.scalar.activation(
                                    out=off_t[ch][:, k, :], in_=ps, func=AF.Identity,
                                    bias=boff_sb[:, ch, k:k + 1], scale=1.0)
                            else:
                                nc.scalar.activation(
                                    out=exp_t[ch][:, k, :], in_=ps, func=AF.Exp,
                                    bias=boff_sb[:, ch, K + k:K + k + 1], scale=1.0)

                # softmax denominators -> rec_all [128, K, LT] fp16
                rc32 = work.tile([128, K, LT], f32, name="rc32", tag="rc32", bufs=2)
                for k in range(K):
                    ps = pss.tile([128, LT], f32, name="psm", tag="psm")
                    for sub in range(NSUB):
                        nc.tensor.matmul(ps[:, sub * 512:(sub + 1) * 512],
                                         lhsT=ones_sb,
                                         rhs=exp_t[0][:, k, sub * 512:(sub + 1) * 512],
                                         start=True, stop=False)
                        nc.tensor.matmul(ps[:, sub * 512:(sub + 1) * 512],
                                         lhsT=ones_sb,
                                         rhs=exp_t[1][:, k, sub * 512:(sub + 1) * 512],
                                         start=False, stop=True)
                    nc.vector.reciprocal_approx_fast(out=rc32[:, k, :], in_=ps)
                rec = work.tile([128, K, LT], f16, name="rec", tag="rec", bufs=1)
                nc.scalar.activation(out=rec, in_=rc32, func=AF.Copy, bias=0.0, scale=1.0)

                # deformable window, all K taps in one op via [[1,K],[1,LT]] views
                y_t = []
                for ch in range(NCH):
                    v = work.tile([128, K, LT], f16, name="v", tag="v", bufs=2)
                    u = work.tile([128, K, LT], f16, name="u", tag="u", bufs=1)
                    mt = work.tile([128, K, LT], f16, name="mt", tag="mt", bufs=1)
                    # merged d1 view for shift j: element (k, t) = d1[(l0+k-2+j) - D1LO + t]
                    # base A-index for k=0: (l0-2+j) - D1LO = j + 4
                    for j in range(JMIN, JMAX):
                        s = j + 4
                        if s % 2 == 0:
                            src, sidx = d1A[ch], s
                        else:
                            src, sidx = d1B[ch], s - 1
                        d1view = ov(src[:, sidx:sidx + 1], K, LT)
                        nc.vector.tensor_scalar(
                            out=u, in0=off_t[ch],
                            scalar1=float(j), scalar2=float(j + 1),
                            op0=alu.max, op1=alu.min)
                        if j == JMIN:
                            nc.vector.tensor_tensor(out=v, in0=u, in1=d1view, op=alu.mult)
                        else:
                            nc.vector.tensor_tensor(out=mt, in0=u, in1=d1view, op=alu.mult)
                            nc.vector.tensor_tensor(out=v, in0=v, in1=mt, op=alu.add)
                    bview = ov(baset[ch][:, 0:1], K, LT)
                    nc.vector.tensor_tensor(out=v, in0=v, in1=bview, op=alu.add)
                    nc.gpsimd.tensor_tensor(out=v, in0=v, in1=exp_t[ch], op=alu.mult)
                    y = work.tile([128, K, LT], f16, name=f"y_{ch}", tag=f"y_{ch}", bufs=2)
                    nc.gpsimd.tensor_tensor(out=y, in0=v, in1=rec, op=alu.mult)
                    y_t.append(y)

                for ch in range(NCH):
                    ps = psf.tile([128, LT], f32, name="pfin", tag="pfin")
                    for sub in range(NSUB):
                        for k in range(K):
                            nc.tensor.matmul(
                                ps[:, sub * 512:(sub + 1) * 512],
                                lhsT=wfin_sb[:, k * NCH + ch, :],
                                rhs=y_t[ch][:, k, sub * 512:(sub + 1) * 512],
                                start=(k == 0), stop=(k == K - 1))
                    og = work.tile([128, LT], f32, name="og", tag="og", bufs=2)
                    nc.scalar.activation(
                        out=og, in_=ps, func=AF.Identity,
                        bias=bfin_sb[:, ch:ch + 1], scale=1.0)
                    nc.sync.dma_start(
                        out=out_d[ch * 128:(ch + 1) * 128, l0:l0 + LT], in_=og)
    return nc


def _get_compiled():
    if "nc" not in _CACHE:
        import concourse.bacc as bacc
        nc = bacc.Bacc()
        _build(nc)
        nc.compile()
        _CACHE["nc"] = nc
    return _CACHE["nc"]


def kernel(x, w_off, b_off, weight, bias):
    x = np.ascontiguousarray(np.asarray(x, dtype=np.float32))
    w_off = np.asarray(w_off, dtype=np.float32)
    b_off = np.asarray(b_off, dtype=np.float32)
    weight = np.asarray(weight, dtype=np.float32)
    bias = np.asarray(bias, dtype=np.float32)

    wofflhsT, wfinlhsT, boffs, bfin, ones_sm = _pack_weights(w_off, b_off, weight, bias)
    nc = _get_compiled()

    from concourse.bass_utils import run_bass_kernel_spmd
    in_maps = []
    for b in range(B):
        in_maps.append({
            "x": np.ascontiguousarray(x[b]),
            "wofflhsT": wofflhsT,
            "wfinlhsT": wfinlhsT,
            "boffs": boffs,
            "bfin": bfin,
            "ones_sm": ones_sm,
        })
    res = run_bass_kernel_spmd(nc, in_maps, core_ids=list(range(B)),
                               trace=TRACE, stitch_traces=TRACE)
    global LAST_EXEC_NS
    if res.exec_time_ns is not None:
        LAST_EXEC_NS = res.exec_time_ns
    if TRACE and res.instructions_and_trace is not None:
        print("trace:", res.instructions_and_trace[1])
        print("per-core scope times:", res.per_core_scope_times)
    out = np.stack([res.results[b]["out"] for b in range(B)], axis=0)
    return out

